# revision 14
# baseline (speedup 1.0000x reference)
"""Trainium2 Bass kernel for ConstantTimeStrideAttention (v2, bf16).

Model (reference):
  qkv = x @ Wqkv + bqkv -> q,k,v per head (B=2, S=2048, DIM=1536, H=12, HD=128)
  per query s: 12 anchors (6 local +-1..3, 4 strided +-5,+-10, 2 global {0,S-1})
  attn = softmax(q . k_anchor * HD^-0.5 + log(group_weight)); out = attn @ v_anchors
  y = concat_heads @ Wout + bout

Sharding: 8 cores = (2 batches) x (4 sequence chunks of 512 queries). No
collectives; each core recomputes its k/v halo + globals from the full x.

v2 layout: the ext token axis is 640 = 5 blocks of 128 cols, one per
attention q-tile (tile sizes 106,106,106,106,88). Block t = [tok 0, tok S-1,
window q0-10 .. q0+115 of tile t (126 tokens)]. Every q-tile's full anchor
set (window +-10 plus both globals) lives inside its own 128-col block, so
scores / AV / denominator are each ONE 128-contraction matmul per tile. A
separate xq tensor carries the plain 512 query tokens for the Q projection
(queries are not contiguous in the block layout).

All matmul operands are bf16 (same 0.417 ns/row PE rate as f32r, lower
per-instruction overhead, half the DMA bytes); PSUM accumulation is fp32.

Per-head attention: 5 score matmuls write column ranges of one PSUM bank
[128,512] -> one exp (ScalarE) -> one mask-multiply (VectorE; the mask
carries softmax group weights on anchor rows, zero elsewhere) -> 5 AV
matmuls + 1 ones-matmul denominator (replicating each query's sum across
all 128 partitions) -> reciprocal + multiply on PSUM evacuation. Emission
interleaves the next block's V/K matmuls and the next head's Q matmuls
between each head's score and AV matmuls so the PE never idles on the
exp->mask cross-engine latency; the out-projection's first chunk fills the
last head's gap, and output chunks then stream with bias-add + DMA out
overlapped behind the remaining matmuls.

DMA rings: scalar (Act HWDGE) carries only startup loads (first V weight
group + consts) plus one 4-slice group mid-kernel, so exp never sits behind
a long issue batch; sync (SP HWDGE) carries xt and the K weight groups
per-f; gpsimd (Pool SWDGE, ~1us fixed issue cost) carries everything else
as 4 coarse slices per group (slice-major host layout [.,4,128,1536]).
"""

import sys

sys.path.insert(0, "/opt/trn_rl_repo")

import numpy as np  # noqa: E402
import ml_dtypes  # noqa: E402

import concourse.bass as bass  # noqa: E402,F401
import concourse.tile as tile  # noqa: E402
from concourse import bacc, mybir  # noqa: E402
from concourse import bass_utils  # noqa: E402

F32 = mybir.dt.float32
BF16 = mybir.dt.bfloat16
NPBF16 = ml_dtypes.bfloat16
EXP = mybir.ActivationFunctionType.Exp

B, S, DIM = 2, 2048, 1536
H, HD = 12, 128
NCORES = 8
SCHUNKS = 4          # sequence chunks per batch
Q = S // SCHUNKS     # 512 queries per core
WIN = 10             # max |anchor offset|
NF = DIM // 128      # 12 contraction chunks
NT = 5               # attention q-tiles per core
TQ = 106             # q-tile size (last tile: 512 - 4*106 = 88)
EXT = NT * 128       # 640 ext cols: 5 blocks of [g0 g1 window126]
OFFS = [-3, -2, -1, 1, 2, 3, -10, -5, 5, 10]
TILES = [(t, t * TQ, min(TQ, Q - t * TQ)) for t in range(NT)]

_CACHE = {}


def _build_program():
    nc = bacc.Bacc("TRN2", target_bir_lowering=False, debug=False)

    # xt in 6 two-fchunk slices [128, 1280] (2.5KB/partition packets)
    xt_d = nc.dram_tensor("xt2", [6, 128, 2 * EXT], BF16, kind="ExternalInput").ap()
    xq_d = nc.dram_tensor("xq4", [4, 128, 3 * Q], BF16, kind="ExternalInput").ap()
    # weights pre-tiled on host as 4 three-fchunk slices per group
    # [group, slice, 128, 1536]: 3KB/partition packets on every ring
    wqkv4_d = nc.dram_tensor(
        "wqkv4", [9, 4, 128, 3 * 512], BF16, kind="ExternalInput"
    ).ap()
    wout4_d = nc.dram_tensor(
        "wout4", [3, 4, 128, 3 * 512], BF16, kind="ExternalInput"
    ).ap()
    # bias columns pre-transposed on host: [:, 0:12]=bq, 12:24=bk, 24:36=bo
    bcol_d = nc.dram_tensor("bcol", [128, 3 * H], F32, kind="ExternalInput").ap()
    # V bias rows pre-broadcast on host to all 128 partitions: [128, 3*512]
    bvb_d = nc.dram_tensor("bvb", [128, 3 * 512], BF16, kind="ExternalInput").ap()
    ones_d = nc.dram_tensor("ones_sq", [128, 128], BF16, kind="ExternalInput").ap()
    mask_d = nc.dram_tensor("mask", [128, Q], BF16, kind="ExternalInput").ap()
    yt_d = nc.dram_tensor("yt", [DIM, Q], BF16, kind="ExternalOutput").ap()

    with tile.TileContext(nc) as tc:
        const = tc.alloc_tile_pool(name="const", bufs=1)
        xt_pool = tc.alloc_tile_pool(name="xt", bufs=1)
        wq_pool = tc.alloc_tile_pool(name="wq", bufs=7)
        qT_pool = tc.alloc_tile_pool(name="qT", bufs=3)
        kT_pool = tc.alloc_tile_pool(name="kT", bufs=12)
        v_pool = tc.alloc_tile_pool(name="v", bufs=15)
        at_pool = tc.alloc_tile_pool(name="at", bufs=1)  # 12 distinct tags
        et_pool = tc.alloc_tile_pool(name="et", bufs=2)
        ptm_pool = tc.alloc_tile_pool(name="ptm", bufs=2)
        rec_pool = tc.alloc_tile_pool(name="rec", bufs=2)
        yt_sb_pool = tc.alloc_tile_pool(name="yt_sb", bufs=2)

        # PSUM: 8 banks total: v1 + k1 + q1 + yt2 + pt1 + av1 + dn1
        v_ps = tc.alloc_tile_pool(name="v_ps", bufs=1, space="PSUM")
        k_ps = tc.alloc_tile_pool(name="k_ps", bufs=1, space="PSUM")
        q_ps = tc.alloc_tile_pool(name="q_ps", bufs=1, space="PSUM")
        yt_ps = tc.alloc_tile_pool(name="yt_ps", bufs=2, space="PSUM")
        pt_ps = tc.alloc_tile_pool(name="pt_ps", bufs=1, space="PSUM")
        av_ps = tc.alloc_tile_pool(name="av_ps", bufs=1, space="PSUM")
        dn_ps = tc.alloc_tile_pool(name="dn_ps", bufs=1, space="PSUM")

        # ---- startup DMAs ----
        xt_t = xt_pool.tile([128, NF * EXT], BF16, tag="xt")

        def load_xt2(s, eng):
            eng.dma_start(xt_t[:, s * 2 * EXT : (s + 1) * 2 * EXT], xt_d[s])

        xt = [xt_t[:, f * EXT : (f + 1) * EXT] for f in range(NF)]

        wq_tiles = {}

        def load_w4_slice(key, g, s, eng, src_d):
            if key not in wq_tiles:
                wq_tiles[key] = wq_pool.tile(
                    [128, NF * 512], BF16, tag="wqg", name="wqg"
                )
            t = wq_tiles[key]
            eng.dma_start(t[:, s * 1536 : (s + 1) * 1536], src_d[g, s])
            return [t[:, f * 512 : (f + 1) * 512] for f in range(NF)]

        def load_w4(g, eng, src_d=wqkv4_d, key=None):
            key = key or ("qkv", g)
            for s in range(4):
                out = load_w4_slice(key, g, s, eng, src_d)
            return out

        wv = [None] * 3
        wk = [None] * 3
        wqg = [None] * 3
        wo = [None] * 3

        # startup interleave: the first V chunk's f-accumulation consumes
        # (xt[f], wv0[f]) pairs in order, fed by both HWDGE rings in parallel:
        #   sync:   xt s0 s1 s2 | g3 s0..s3
        #   scalar: g6 s0, xt s3, g6 s1, xt s4, g6 s2, xt s5, g6 s3 | consts
        load_xt2(0, nc.sync)
        wv[0] = load_w4_slice(("qkv", 6), 6, 0, nc.scalar, wqkv4_d)
        load_xt2(1, nc.sync)
        load_xt2(3, nc.scalar)
        load_w4_slice(("qkv", 6), 6, 1, nc.scalar, wqkv4_d)
        load_xt2(2, nc.sync)
        load_xt2(4, nc.scalar)
        load_w4_slice(("qkv", 6), 6, 2, nc.scalar, wqkv4_d)
        load_xt2(5, nc.scalar)
        load_w4_slice(("qkv", 6), 6, 3, nc.scalar, wqkv4_d)
        wk[0] = load_w4(3, nc.sync)

        # consts: small ones on scalar, big bvb on gpsimd
        ones_t = const.tile([128, 128], BF16, tag="ones")
        nc.scalar.dma_start(ones_t[:], ones_d[:])
        bcol_t = const.tile([128, 3 * H], F32, tag="bcol")
        nc.scalar.dma_start(bcol_t[:], bcol_d[:])
        mask_t = const.tile([128, Q], BF16, tag="mask")
        nc.scalar.dma_start(mask_t[:], mask_d[:])
        bvb_full = const.tile([128, 3 * 512], BF16, tag="bvb")
        nc.gpsimd.dma_start(bvb_full[:], bvb_d[:])
        bq_t = [bcol_t[:, i : i + 1] for i in range(H)]
        bk_t = [bcol_t[:, H + i : H + i + 1] for i in range(H)]
        bo_t = [bcol_t[:, 2 * H + i : 2 * H + i + 1] for i in range(H)]
        bvb_t = [bvb_full[:, g * 512 : (g + 1) * 512] for g in range(3)]

        # gpsimd ring (fastest): next block's K weights first, then Q-blk0
        wk[1] = load_w4(4, nc.gpsimd)
        wqg[0] = load_w4(0, nc.gpsimd)
        xq_t = xt_pool.tile([128, NF * Q], BF16, tag="xq")
        xq = [xq_t[:, f * Q : (f + 1) * Q] for f in range(NF)]

        def load_xq():
            for s in range(4):
                nc.gpsimd.dma_start(xq_t[:, s * 1536 : (s + 1) * 1536], xq_d[s])

        qT = [None] * H
        kT = [None] * H
        vv = [[None] * 3 for _ in range(NT)]   # [block][group]
        at = [None] * H

        def emit_v_chunk(g, c):
            ps = v_ps.tile([128, 512], F32)
            wt = wv[g]
            for f in range(NF):
                nc.tensor.matmul(
                    ps[:], xt[f][:, c * 128 : (c + 1) * 128], wt[f][:],
                    start=(f == 0), stop=(f == NF - 1),
                )
            sb = v_pool.tile([128, 512], BF16, tag="v")
            nc.vector.tensor_add(sb[:], ps[:], bvb_t[g][:])
            vv[c][g] = sb

        def emit_k_half(hcur, j):
            # half j of kT[hcur]: ext cols [j*320, (j+1)*320)
            hx = hcur % 4
            wt = wk[hcur // 4]
            if j == 0:
                kT[hcur] = kT_pool.tile([128, EXT], BF16, tag="kT", name="kT")
            ps = k_ps.tile([128, 320], F32)
            for f in range(NF):
                nc.tensor.matmul(
                    ps[:], wt[f][:, hx * 128 : (hx + 1) * 128],
                    xt[f][:, j * 320 : (j + 1) * 320],
                    start=(f == 0), stop=(f == NF - 1),
                )
            nc.vector.tensor_scalar_add(
                kT[hcur][:, j * 320 : (j + 1) * 320], ps[:], bk_t[hcur][:]
            )

        def emit_q(hcur):
            # blk0 reads queries from xt block interiors (query q of tile t
            # sits at block col 12+q-TQ*t), so Q(0..3) need no xq tensor and
            # the startup-critical DMA set shrinks by 1.5MB; later blocks use
            # the contiguous xq (arrives mid-kernel on the gpsimd ring).
            hx = hcur % 4
            wt = wqg[hcur // 4]
            ps = q_ps.tile([128, Q], F32)
            if hcur < 4:
                for t, qs, qsz in TILES:
                    c0 = t * 128 + 12
                    for f in range(NF):
                        nc.tensor.matmul(
                            ps[:, qs : qs + qsz],
                            wt[f][:, hx * 128 : (hx + 1) * 128],
                            xt[f][:, c0 : c0 + qsz],
                            start=(f == 0), stop=(f == NF - 1),
                        )
            else:
                for f in range(NF):
                    nc.tensor.matmul(
                        ps[:], wt[f][:, hx * 128 : (hx + 1) * 128], xq[f][:],
                        start=(f == 0), stop=(f == NF - 1),
                    )
            sb = qT_pool.tile([128, Q], BF16, tag="qT")
            nc.vector.tensor_scalar_add(sb[:], ps[:], bq_t[hcur][:])
            qT[hcur] = sb

        def emit_scores(h):
            pt = pt_ps.tile([128, Q], F32)
            for t, qs, qsz in TILES:
                nc.tensor.matmul(
                    pt[:, qs : qs + qsz], kT[h][:, t * 128 : (t + 1) * 128],
                    qT[h][:, qs : qs + qsz], start=True, stop=True,
                )
            et = et_pool.tile([128, Q], BF16, tag="et")
            nc.scalar.activation(et[:], pt[:], EXP)
            ptm = ptm_pool.tile([128, Q], BF16, tag="ptm")
            nc.vector.tensor_mul(ptm[:], et[:], mask_t[:])
            return ptm

        def emit_av(h, ptm):
            g, hx = h // 4, h % 4
            av = av_ps.tile([128, Q], F32)
            for t, qs, qsz in TILES:
                nc.tensor.matmul(
                    av[:, qs : qs + qsz],
                    vv[t][g][:, hx * 128 : (hx + 1) * 128],
                    ptm[:, qs : qs + qsz], start=True, stop=True,
                )
            dn = dn_ps.tile([128, Q], F32)
            nc.tensor.matmul(dn[:], ones_t[:], ptm[:], start=True, stop=True)
            rec = rec_pool.tile([128, Q], F32, tag="rec")
            nc.vector.reciprocal_approx_fast(rec[:], dn[:])
            sb = at_pool.tile([128, Q], BF16, tag=f"at{h}")
            nc.vector.tensor_mul(sb[:], av[:], rec[:])
            at[h] = sb

        _oc_ps = {}

        def emit_oproj(oc, f_lo, f_hi):
            # accumulate f in [f_lo, f_hi) of output chunk oc into its psum
            og, ox = oc // 4, oc % 4
            if oc not in _oc_ps:
                _oc_ps[oc] = yt_ps.tile([128, Q], F32, tag="yt_ps", name="yt_ps")
            ps = _oc_ps[oc]
            for f in range(f_lo, f_hi):
                nc.tensor.matmul(
                    ps[:], wo[og][f][:, ox * 128 : (ox + 1) * 128], at[f][:],
                    start=(f == 0), stop=(f == NF - 1),
                )
            if f_hi == NF:
                sb = yt_sb_pool.tile([128, Q], BF16, tag="yt")
                nc.vector.tensor_scalar_add(sb[:], ps[:], bo_t[oc][:])
                eng = (nc.sync, nc.scalar, nc.gpsimd)[oc % 3]
                eng.dma_start(yt_d[oc * 128 : (oc + 1) * 128, :], sb[:])
                del _oc_ps[oc]

        # ---- prologue: V(0) and K(0) interleaved (psum evac overlap), Q(0)
        prologue = [("v", 0, 0), ("v", 0, 1), ("k", 0, 0), ("v", 0, 2),
                    ("k", 0, 1), ("v", 0, 3), ("k", 1, 0), ("v", 0, 4),
                    ("k", 1, 1), ("k", 2, 0), ("k", 2, 1), ("k", 3, 0),
                    ("k", 3, 1)]
        for kind, a, c in prologue:
            if kind == "v":
                emit_v_chunk(a, c)
            else:
                emit_k_half(a, c)
        emit_q(0)

        # filler units for block bn, consumed across the previous block's heads
        def blk_units(bn):
            return [("k", 4 * bn + 0, 0), ("k", 4 * bn + 0, 1),
                    ("k", 4 * bn + 1, 0), ("k", 4 * bn + 1, 1),
                    ("k", 4 * bn + 2, 0), ("k", 4 * bn + 2, 1),
                    ("k", 4 * bn + 3, 0), ("k", 4 * bn + 3, 1),
                    ("v", bn, 0), ("v", bn, 1), ("v", bn, 2),
                    ("v", bn, 3), ("v", bn, 4)]

        UNITS_PER_HEAD = [3, 3, 3, 4]

        for h in range(H):
            b2, i = h // 4, h % 4
            ptm = emit_scores(h)
            # stream upcoming weight groups (post-exp so Act isn't blocked)
            if h == 0:
                wv[1] = load_w4(7, nc.sync)
                wqg[1] = load_w4(1, nc.gpsimd)
                load_xq()
            elif h == 2:
                wv[2] = load_w4(8, nc.scalar)
                wk[2] = load_w4(5, nc.gpsimd)
                wqg[2] = load_w4(2, nc.gpsimd)
            elif h == 4:
                wo[0] = load_w4(0, nc.scalar, src_d=wout4_d, key=("out", 0))
                wo[1] = load_w4(1, nc.sync, src_d=wout4_d, key=("out", 1))
                wo[2] = load_w4(2, nc.gpsimd, src_d=wout4_d, key=("out", 2))
            # fill the exp->mask latency with the next head's Q projection
            if h + 1 < H:
                emit_q(h + 1)
            else:
                emit_oproj(0, 0, 8)
            emit_av(h, ptm)
            # next-block V/K work between heads (also covers qT evacuation)
            if b2 < 2:
                units = blk_units(b2 + 1)
                lo = sum(UNITS_PER_HEAD[:i])
                for kind, a, c in units[lo : lo + UNITS_PER_HEAD[i]]:
                    if kind == "v":
                        emit_v_chunk(a, c)
                    else:
                        emit_k_half(a, c)

        # ---- output projection ----
        emit_oproj(0, 8, NF)
        for oc in range(1, 12):
            emit_oproj(oc, 0, NF)

        for p in (dn_ps, av_ps, pt_ps, yt_ps, q_ps, k_ps, v_ps,
                  yt_sb_pool, rec_pool, ptm_pool, et_pool, at_pool, v_pool,
                  kT_pool, qT_pool, wq_pool, xt_pool, const):
            p.release()

    nc.compile()
    return nc


def _softmax(v):
    e = np.exp(v - v.max())
    return e / e.sum()


def _build_mask(r0, gw):
    """Per-core [128, 512] mask: routes softmax group weights onto the anchor
    rows of each query's block-local transposed score column."""
    m = np.zeros((128, Q), np.float32)
    wts = [gw[0]] * 6 + [gw[1]] * 4
    for q in range(Q):
        t = min(q // TQ, NT - 1)
        lo = r0 + TQ * t - WIN
        for off, w in zip(OFFS, wts):
            tok = min(max(r0 + q + off, 0), S - 1)
            row = 2 + (tok - lo)
            assert 2 <= row < 128, (q, off, tok, row)
            m[row, q] += w
        m[0, q] += gw[2]   # token 0
        m[1, q] += gw[2]   # token S-1
    return m


def _slicemajor(a):
    """[G, NF, 128, 512] -> [G, 4, 128, 1536]: 3 consecutive f-chunks per
    slice, partition-major inside each slice (single contiguous DMA)."""
    g = a.shape[0]
    return np.ascontiguousarray(
        a.reshape(g, 4, 3, 128, 512).transpose(0, 1, 3, 2, 4).reshape(
            g, 4, 128, 3 * 512
        )
    )


def _prepare_in_maps(x, wqkv, bqkv, wout, bout, group_scale):
    scale = HD ** -0.5
    wqkv_m = np.array(wqkv, np.float32, copy=True)
    wqkv_m[:, :DIM] *= scale
    # pre-tile: [9 groups, 12 fchunks, 128, 512] contiguous per [128,512] tile
    wqkv_t = np.ascontiguousarray(
        wqkv_m.reshape(NF, 128, 9, 512).transpose(2, 0, 1, 3)
    ).astype(NPBF16)  # [9, NF, 128, 512]
    bqkv_m = np.array(bqkv, np.float32, copy=True)
    bqkv_m[:DIM] *= scale
    gw = _softmax(np.asarray(group_scale, np.float64))

    # bias columns [128, 36]: q heads, k heads, then out-proj chunks
    bcol = np.concatenate(
        [
            bqkv_m[:DIM].reshape(H, 128),
            bqkv_m[DIM : 2 * DIM].reshape(H, 128),
            np.asarray(bout, np.float32).reshape(H, 128),
        ],
        axis=0,
    ).T.astype(np.float32).copy()  # [128, 36]
    bvb = np.broadcast_to(bqkv_m[2 * DIM :][None, :], (128, 3 * 512)).astype(
        NPBF16
    ).copy()
    wout_t = np.ascontiguousarray(
        np.asarray(wout, np.float32).reshape(NF, 128, 3, 512).transpose(2, 0, 1, 3)
    ).astype(NPBF16)
    wqkv4 = _slicemajor(wqkv_t)
    wout4 = _slicemajor(wout_t)
    ones_sq = np.ones((128, 128), NPBF16)

    in_maps = []
    for core in range(NCORES):
        b, sc = divmod(core, SCHUNKS)
        r0 = sc * Q
        tok_ids = np.concatenate(
            [
                np.concatenate(
                    [
                        [0, S - 1],
                        np.clip(
                            np.arange(r0 + TQ * t - WIN, r0 + TQ * t - WIN + 126),
                            0, S - 1,
                        ),
                    ]
                )
                for t in range(NT)
            ]
        ).astype(np.int64)
        x_ext_t = np.ascontiguousarray(x[b, tok_ids, :].T).astype(NPBF16)
        xt2 = np.ascontiguousarray(
            x_ext_t.reshape(6, 2, 128, EXT).transpose(0, 2, 1, 3).reshape(
                6, 128, 2 * EXT
            )
        )
        xq_t = np.ascontiguousarray(x[b, r0 : r0 + Q, :].T).astype(NPBF16)
        xq4 = _slicemajor(xq_t.reshape(1, NF, 128, Q))[0]
        mask = _build_mask(r0, gw).astype(NPBF16)
        in_maps.append(
            {
                "xt2": xt2,
                "xq4": xq4,
                "wqkv4": wqkv4,
                "wout4": wout4,
                "bcol": bcol,
                "bvb": bvb,
                "ones_sq": ones_sq,
                "mask": mask,
            }
        )
    return in_maps


def get_program():
    if "nc" not in _CACHE:
        _CACHE["nc"] = _build_program()
    return _CACHE["nc"]


def run(inputs, **spmd_kwargs):
    """Run the SPMD kernel; returns (y [B,S,DIM] fp32, BassKernelResults)."""
    x = np.asarray(inputs["x"], np.float32)
    in_maps = _prepare_in_maps(
        x,
        np.asarray(inputs["Wqkv"], np.float32),
        np.asarray(inputs["bqkv"], np.float32),
        np.asarray(inputs["Wout"], np.float32),
        np.asarray(inputs["bout"], np.float32),
        np.asarray(inputs["group_scale"], np.float32),
    )
    nc = get_program()
    res = bass_utils.run_bass_kernel_spmd(
        nc, in_maps, core_ids=list(range(NCORES)), **spmd_kwargs
    )
    y = np.empty((B, S, DIM), np.float32)
    for core in range(NCORES):
        b, sc = divmod(core, SCHUNKS)
        y[b, sc * Q : (sc + 1) * Q, :] = res.results[core]["yt"].T.astype(np.float32)
    return y, res


def kernel(**inputs):
    y, _ = run(inputs)
    return y


# revision 15
# speedup vs baseline: 1.1076x; 1.1076x over previous
"""Trainium2 Bass kernel for ConstantTimeStrideAttention (v2, bf16).

Model (reference):
  qkv = x @ Wqkv + bqkv -> q,k,v per head (B=2, S=2048, DIM=1536, H=12, HD=128)
  per query s: 12 anchors (6 local +-1..3, 4 strided +-5,+-10, 2 global {0,S-1})
  attn = softmax(q . k_anchor * HD^-0.5 + log(group_weight)); out = attn @ v_anchors
  y = concat_heads @ Wout + bout

Sharding: 8 cores = (2 batches) x (4 sequence chunks of 512 queries). No
collectives; each core recomputes its k/v halo + globals from the full x.

v2 layout: the ext token axis is 640 = 5 blocks of 128 cols, one per
attention q-tile (tile sizes 106,106,106,106,88). Block t = [tok 0, tok S-1,
window q0-10 .. q0+115 of tile t (126 tokens)]. Every q-tile's full anchor
set (window +-10 plus both globals) lives inside its own 128-col block, so
scores / AV / denominator are each ONE 128-contraction matmul per tile. A
separate xq tensor carries the plain 512 query tokens for the Q projection
(queries are not contiguous in the block layout).

All matmul operands are bf16 (same 0.417 ns/row PE rate as f32r, lower
per-instruction overhead, half the DMA bytes); PSUM accumulation is fp32.

Per-head attention: 5 score matmuls write column ranges of one PSUM bank
[128,512] -> one exp (ScalarE) -> one mask-multiply (VectorE; the mask
carries softmax group weights on anchor rows, zero elsewhere) -> 5 AV
matmuls + 1 ones-matmul denominator (replicating each query's sum across
all 128 partitions) -> reciprocal + multiply on PSUM evacuation. Emission
interleaves the next block's V/K matmuls and the next head's Q matmuls
between each head's score and AV matmuls so the PE never idles on the
exp->mask cross-engine latency; the out-projection's first chunk fills the
last head's gap, and output chunks then stream with bias-add + DMA out
overlapped behind the remaining matmuls.

DMA rings: scalar (Act HWDGE) carries only startup loads (first V weight
group + consts) plus one 4-slice group mid-kernel, so exp never sits behind
a long issue batch; sync (SP HWDGE) carries xt and the K weight groups
per-f; gpsimd (Pool SWDGE, ~1us fixed issue cost) carries everything else
as 4 coarse slices per group (slice-major host layout [.,4,128,1536]).
"""

import sys

sys.path.insert(0, "/opt/trn_rl_repo")

import numpy as np  # noqa: E402
import ml_dtypes  # noqa: E402

import concourse.bass as bass  # noqa: E402,F401
import concourse.tile as tile  # noqa: E402
from concourse import bacc, mybir  # noqa: E402
from concourse import bass_utils  # noqa: E402

F32 = mybir.dt.float32
BF16 = mybir.dt.bfloat16
NPBF16 = ml_dtypes.bfloat16
EXP = mybir.ActivationFunctionType.Exp

B, S, DIM = 2, 2048, 1536
H, HD = 12, 128
NCORES = 8
SCHUNKS = 4          # sequence chunks per batch
Q = S // SCHUNKS     # 512 queries per core
WIN = 10             # max |anchor offset|
NF = DIM // 128      # 12 contraction chunks
NT = 5               # attention q-tiles per core
TQ = 106             # q-tile size (last tile: 512 - 4*106 = 88)
EXT = NT * 128       # 640 ext cols: 5 blocks of [g0 g1 window126]
OFFS = [-3, -2, -1, 1, 2, 3, -10, -5, 5, 10]
TILES = [(t, t * TQ, min(TQ, Q - t * TQ)) for t in range(NT)]

_CACHE = {}


def _build_program():
    nc = bacc.Bacc("TRN2", target_bir_lowering=False, debug=False)

    # xt in 6 two-fchunk slices [128, 1280] (2.5KB/partition packets)
    xt_d = nc.dram_tensor("xt2", [6, 128, 2 * EXT], BF16, kind="ExternalInput").ap()
    xq_d = nc.dram_tensor("xq4", [4, 128, 3 * Q], BF16, kind="ExternalInput").ap()
    # weights pre-tiled on host as 4 three-fchunk slices per group
    # [group, slice, 128, 1536]: 3KB/partition packets on every ring
    wqkv4_d = nc.dram_tensor(
        "wqkv4", [9, 4, 128, 3 * 512], BF16, kind="ExternalInput"
    ).ap()
    wout4_d = nc.dram_tensor(
        "wout4", [3, 4, 128, 3 * 512], BF16, kind="ExternalInput"
    ).ap()
    # bias columns pre-transposed on host: [:, 0:12]=bq, 12:24=bk, 24:36=bo
    bcol_d = nc.dram_tensor("bcol", [128, 3 * H], F32, kind="ExternalInput").ap()
    # V bias rows pre-broadcast on host to all 128 partitions: [128, 3*512]
    bvb_d = nc.dram_tensor("bvb", [128, 3 * 512], BF16, kind="ExternalInput").ap()
    ones_d = nc.dram_tensor("ones_sq", [128, 128], BF16, kind="ExternalInput").ap()
    mask_d = nc.dram_tensor("mask", [128, Q], BF16, kind="ExternalInput").ap()
    yt_d = nc.dram_tensor("yt", [DIM, Q], BF16, kind="ExternalOutput").ap()

    with tile.TileContext(nc) as tc:
        const = tc.alloc_tile_pool(name="const", bufs=1)
        xt_pool = tc.alloc_tile_pool(name="xt", bufs=1)
        wq_pool = tc.alloc_tile_pool(name="wq", bufs=7)
        qT_pool = tc.alloc_tile_pool(name="qT", bufs=3)
        kT_pool = tc.alloc_tile_pool(name="kT", bufs=12)
        v_pool = tc.alloc_tile_pool(name="v", bufs=15)
        at_pool = tc.alloc_tile_pool(name="at", bufs=1)  # 12 distinct tags
        et_pool = tc.alloc_tile_pool(name="et", bufs=2)
        ptm_pool = tc.alloc_tile_pool(name="ptm", bufs=2)
        rec_pool = tc.alloc_tile_pool(name="rec", bufs=2)
        yt_sb_pool = tc.alloc_tile_pool(name="yt_sb", bufs=2)

        # PSUM: 8 banks total: v1 + k1 + q1 + yt2 + pt1 + av1 + dn1
        v_ps = tc.alloc_tile_pool(name="v_ps", bufs=1, space="PSUM")
        k_ps = tc.alloc_tile_pool(name="k_ps", bufs=1, space="PSUM")
        q_ps = tc.alloc_tile_pool(name="q_ps", bufs=1, space="PSUM")
        yt_ps = tc.alloc_tile_pool(name="yt_ps", bufs=2, space="PSUM")
        pt_ps = tc.alloc_tile_pool(name="pt_ps", bufs=1, space="PSUM")
        av_ps = tc.alloc_tile_pool(name="av_ps", bufs=1, space="PSUM")
        dn_ps = tc.alloc_tile_pool(name="dn_ps", bufs=1, space="PSUM")

        # ---- startup DMAs ----
        xt_t = xt_pool.tile([128, NF * EXT], BF16, tag="xt")

        def load_xt2(s, eng):
            eng.dma_start(xt_t[:, s * 2 * EXT : (s + 1) * 2 * EXT], xt_d[s])

        xt = [xt_t[:, f * EXT : (f + 1) * EXT] for f in range(NF)]

        wq_tiles = {}

        def load_w4_slice(key, g, s, eng, src_d):
            if key not in wq_tiles:
                wq_tiles[key] = wq_pool.tile(
                    [128, NF * 512], BF16, tag="wqg", name="wqg"
                )
            t = wq_tiles[key]
            eng.dma_start(t[:, s * 1536 : (s + 1) * 1536], src_d[g, s])
            return [t[:, f * 512 : (f + 1) * 512] for f in range(NF)]

        def load_w4(g, eng, src_d=wqkv4_d, key=None):
            key = key or ("qkv", g)
            for s in range(4):
                out = load_w4_slice(key, g, s, eng, src_d)
            return out

        wv = [None] * 3
        wk = [None] * 3
        wqg = [None] * 3
        wo = [None] * 3

        # startup: the SWDGE ring is ~2x each HWDGE ring, so the
        # startup-critical weight groups (g6 for V(0), g3 for K(0), g0 for
        # Q(0), g4 for the K(1) fillers) all stream there; the HWDGE rings
        # carry one xt half each plus small consts.
        wv[0] = load_w4(6, nc.gpsimd)
        wk[0] = load_w4(3, nc.gpsimd)
        load_xt2(0, nc.sync)
        load_xt2(1, nc.sync)
        load_xt2(2, nc.sync)
        load_xt2(3, nc.scalar)
        load_xt2(4, nc.scalar)
        load_xt2(5, nc.scalar)
        wqg[0] = load_w4(0, nc.gpsimd)
        wk[1] = load_w4(4, nc.gpsimd)

        ones_t = const.tile([128, 128], BF16, tag="ones")
        nc.scalar.dma_start(ones_t[:], ones_d[:])
        bcol_t = const.tile([128, 3 * H], F32, tag="bcol")
        nc.scalar.dma_start(bcol_t[:], bcol_d[:])
        mask_t = const.tile([128, Q], BF16, tag="mask")
        nc.scalar.dma_start(mask_t[:], mask_d[:])
        bvb_full = const.tile([128, 3 * 512], BF16, tag="bvb")
        nc.sync.dma_start(bvb_full[:], bvb_d[:])
        bq_t = [bcol_t[:, i : i + 1] for i in range(H)]
        bk_t = [bcol_t[:, H + i : H + i + 1] for i in range(H)]
        bo_t = [bcol_t[:, 2 * H + i : 2 * H + i + 1] for i in range(H)]
        bvb_t = [bvb_full[:, g * 512 : (g + 1) * 512] for g in range(3)]

        xq_t = xt_pool.tile([128, NF * Q], BF16, tag="xq")
        xq = [xq_t[:, f * Q : (f + 1) * Q] for f in range(NF)]

        def load_xq():
            for s in range(4):
                nc.gpsimd.dma_start(xq_t[:, s * 1536 : (s + 1) * 1536], xq_d[s])

        qT = [None] * H
        kT = [None] * H
        vv = [[None] * 3 for _ in range(NT)]   # [block][group]
        at = [None] * H

        def emit_v_chunk(g, c):
            ps = v_ps.tile([128, 512], F32)
            wt = wv[g]
            for f in range(NF):
                nc.tensor.matmul(
                    ps[:], xt[f][:, c * 128 : (c + 1) * 128], wt[f][:],
                    start=(f == 0), stop=(f == NF - 1),
                )
            sb = v_pool.tile([128, 512], BF16, tag="v")
            nc.vector.tensor_add(sb[:], ps[:], bvb_t[g][:])
            vv[c][g] = sb

        def emit_k_half(hcur, j):
            # half j of kT[hcur]: ext cols [j*320, (j+1)*320)
            hx = hcur % 4
            wt = wk[hcur // 4]
            if j == 0:
                kT[hcur] = kT_pool.tile([128, EXT], BF16, tag="kT", name="kT")
            ps = k_ps.tile([128, 320], F32)
            for f in range(NF):
                nc.tensor.matmul(
                    ps[:], wt[f][:, hx * 128 : (hx + 1) * 128],
                    xt[f][:, j * 320 : (j + 1) * 320],
                    start=(f == 0), stop=(f == NF - 1),
                )
            nc.vector.tensor_scalar_add(
                kT[hcur][:, j * 320 : (j + 1) * 320], ps[:], bk_t[hcur][:]
            )

        def emit_q(hcur):
            # blk0 reads queries from xt block interiors (query q of tile t
            # sits at block col 12+q-TQ*t), so Q(0..3) need no xq tensor and
            # the startup-critical DMA set shrinks by 1.5MB; later blocks use
            # the contiguous xq (arrives mid-kernel on the gpsimd ring).
            hx = hcur % 4
            wt = wqg[hcur // 4]
            ps = q_ps.tile([128, Q], F32)
            if hcur < 4:
                for t, qs, qsz in TILES:
                    c0 = t * 128 + 12
                    for f in range(NF):
                        nc.tensor.matmul(
                            ps[:, qs : qs + qsz],
                            wt[f][:, hx * 128 : (hx + 1) * 128],
                            xt[f][:, c0 : c0 + qsz],
                            start=(f == 0), stop=(f == NF - 1),
                        )
            else:
                for f in range(NF):
                    nc.tensor.matmul(
                        ps[:], wt[f][:, hx * 128 : (hx + 1) * 128], xq[f][:],
                        start=(f == 0), stop=(f == NF - 1),
                    )
            sb = qT_pool.tile([128, Q], BF16, tag="qT")
            nc.vector.tensor_scalar_add(sb[:], ps[:], bq_t[hcur][:])
            qT[hcur] = sb

        def emit_scores(h):
            pt = pt_ps.tile([128, Q], F32)
            for t, qs, qsz in TILES:
                nc.tensor.matmul(
                    pt[:, qs : qs + qsz], kT[h][:, t * 128 : (t + 1) * 128],
                    qT[h][:, qs : qs + qsz], start=True, stop=True,
                )
            et = et_pool.tile([128, Q], BF16, tag="et")
            nc.scalar.activation(et[:], pt[:], EXP)
            ptm = ptm_pool.tile([128, Q], BF16, tag="ptm")
            nc.vector.tensor_mul(ptm[:], et[:], mask_t[:])
            return ptm

        def emit_av(h, ptm):
            g, hx = h // 4, h % 4
            av = av_ps.tile([128, Q], F32)
            for t, qs, qsz in TILES:
                nc.tensor.matmul(
                    av[:, qs : qs + qsz],
                    vv[t][g][:, hx * 128 : (hx + 1) * 128],
                    ptm[:, qs : qs + qsz], start=True, stop=True,
                )
            dn = dn_ps.tile([128, Q], F32)
            nc.tensor.matmul(dn[:], ones_t[:], ptm[:], start=True, stop=True)
            rec = rec_pool.tile([128, Q], F32, tag="rec")
            nc.vector.reciprocal_approx_fast(rec[:], dn[:])
            sb = at_pool.tile([128, Q], BF16, tag=f"at{h}")
            nc.vector.tensor_mul(sb[:], av[:], rec[:])
            at[h] = sb

        _oc_ps = {}

        def emit_oproj(oc, f_lo, f_hi):
            # accumulate f in [f_lo, f_hi) of output chunk oc into its psum
            og, ox = oc // 4, oc % 4
            if oc not in _oc_ps:
                _oc_ps[oc] = yt_ps.tile([128, Q], F32, tag="yt_ps", name="yt_ps")
            ps = _oc_ps[oc]
            for f in range(f_lo, f_hi):
                nc.tensor.matmul(
                    ps[:], wo[og][f][:, ox * 128 : (ox + 1) * 128], at[f][:],
                    start=(f == 0), stop=(f == NF - 1),
                )
            if f_hi == NF:
                sb = yt_sb_pool.tile([128, Q], BF16, tag="yt")
                nc.vector.tensor_scalar_add(sb[:], ps[:], bo_t[oc][:])
                eng = (nc.sync, nc.scalar, nc.gpsimd)[oc % 3]
                eng.dma_start(yt_d[oc * 128 : (oc + 1) * 128, :], sb[:])
                del _oc_ps[oc]

        # ---- prologue: V(0) and K(0) interleaved (psum evac overlap), Q(0)
        prologue = [("v", 0, 0), ("v", 0, 1), ("k", 0, 0), ("v", 0, 2),
                    ("k", 0, 1), ("v", 0, 3), ("k", 1, 0), ("v", 0, 4),
                    ("k", 1, 1), ("k", 2, 0), ("k", 2, 1), ("k", 3, 0),
                    ("k", 3, 1)]
        for kind, a, c in prologue:
            if kind == "v":
                emit_v_chunk(a, c)
            else:
                emit_k_half(a, c)
        emit_q(0)

        # filler units for block bn, consumed across the previous block's heads
        def blk_units(bn):
            return [("k", 4 * bn + 0, 0), ("k", 4 * bn + 0, 1),
                    ("k", 4 * bn + 1, 0), ("k", 4 * bn + 1, 1),
                    ("k", 4 * bn + 2, 0), ("k", 4 * bn + 2, 1),
                    ("k", 4 * bn + 3, 0), ("k", 4 * bn + 3, 1),
                    ("v", bn, 0), ("v", bn, 1), ("v", bn, 2),
                    ("v", bn, 3), ("v", bn, 4)]

        UNITS_PER_HEAD = [3, 3, 3, 4]

        for h in range(H):
            b2, i = h // 4, h % 4
            ptm = emit_scores(h)
            # stream upcoming weight groups (post-exp so Act isn't blocked)
            if h == 0:
                wv[1] = load_w4(7, nc.sync)
                wqg[1] = load_w4(1, nc.gpsimd)
                load_xq()
            elif h == 2:
                wv[2] = load_w4(8, nc.scalar)
                wk[2] = load_w4(5, nc.gpsimd)
                wqg[2] = load_w4(2, nc.gpsimd)
            elif h == 4:
                wo[0] = load_w4(0, nc.scalar, src_d=wout4_d, key=("out", 0))
                wo[1] = load_w4(1, nc.sync, src_d=wout4_d, key=("out", 1))
                wo[2] = load_w4(2, nc.gpsimd, src_d=wout4_d, key=("out", 2))
            # fill the exp->mask latency with the next head's Q projection
            if h + 1 < H:
                emit_q(h + 1)
            else:
                emit_oproj(0, 0, 8)
            emit_av(h, ptm)
            # next-block V/K work between heads (also covers qT evacuation)
            if b2 < 2:
                units = blk_units(b2 + 1)
                lo = sum(UNITS_PER_HEAD[:i])
                for kind, a, c in units[lo : lo + UNITS_PER_HEAD[i]]:
                    if kind == "v":
                        emit_v_chunk(a, c)
                    else:
                        emit_k_half(a, c)

        # ---- output projection ----
        emit_oproj(0, 8, NF)
        for oc in range(1, 12):
            emit_oproj(oc, 0, NF)

        for p in (dn_ps, av_ps, pt_ps, yt_ps, q_ps, k_ps, v_ps,
                  yt_sb_pool, rec_pool, ptm_pool, et_pool, at_pool, v_pool,
                  kT_pool, qT_pool, wq_pool, xt_pool, const):
            p.release()

    nc.compile()
    return nc


def _softmax(v):
    e = np.exp(v - v.max())
    return e / e.sum()


def _build_mask(r0, gw):
    """Per-core [128, 512] mask: routes softmax group weights onto the anchor
    rows of each query's block-local transposed score column."""
    m = np.zeros((128, Q), np.float32)
    wts = [gw[0]] * 6 + [gw[1]] * 4
    for q in range(Q):
        t = min(q // TQ, NT - 1)
        lo = r0 + TQ * t - WIN
        for off, w in zip(OFFS, wts):
            tok = min(max(r0 + q + off, 0), S - 1)
            row = 2 + (tok - lo)
            assert 2 <= row < 128, (q, off, tok, row)
            m[row, q] += w
        m[0, q] += gw[2]   # token 0
        m[1, q] += gw[2]   # token S-1
    return m


def _slicemajor(a):
    """[G, NF, 128, 512] -> [G, 4, 128, 1536]: 3 consecutive f-chunks per
    slice, partition-major inside each slice (single contiguous DMA)."""
    g = a.shape[0]
    return np.ascontiguousarray(
        a.reshape(g, 4, 3, 128, 512).transpose(0, 1, 3, 2, 4).reshape(
            g, 4, 128, 3 * 512
        )
    )


def _prepare_in_maps(x, wqkv, bqkv, wout, bout, group_scale):
    scale = HD ** -0.5
    wqkv_m = np.array(wqkv, np.float32, copy=True)
    wqkv_m[:, :DIM] *= scale
    # pre-tile: [9 groups, 12 fchunks, 128, 512] contiguous per [128,512] tile
    wqkv_t = np.ascontiguousarray(
        wqkv_m.reshape(NF, 128, 9, 512).transpose(2, 0, 1, 3)
    ).astype(NPBF16)  # [9, NF, 128, 512]
    bqkv_m = np.array(bqkv, np.float32, copy=True)
    bqkv_m[:DIM] *= scale
    gw = _softmax(np.asarray(group_scale, np.float64))

    # bias columns [128, 36]: q heads, k heads, then out-proj chunks
    bcol = np.concatenate(
        [
            bqkv_m[:DIM].reshape(H, 128),
            bqkv_m[DIM : 2 * DIM].reshape(H, 128),
            np.asarray(bout, np.float32).reshape(H, 128),
        ],
        axis=0,
    ).T.astype(np.float32).copy()  # [128, 36]
    bvb = np.broadcast_to(bqkv_m[2 * DIM :][None, :], (128, 3 * 512)).astype(
        NPBF16
    ).copy()
    wout_t = np.ascontiguousarray(
        np.asarray(wout, np.float32).reshape(NF, 128, 3, 512).transpose(2, 0, 1, 3)
    ).astype(NPBF16)
    wqkv4 = _slicemajor(wqkv_t)
    wout4 = _slicemajor(wout_t)
    ones_sq = np.ones((128, 128), NPBF16)

    in_maps = []
    for core in range(NCORES):
        b, sc = divmod(core, SCHUNKS)
        r0 = sc * Q
        tok_ids = np.concatenate(
            [
                np.concatenate(
                    [
                        [0, S - 1],
                        np.clip(
                            np.arange(r0 + TQ * t - WIN, r0 + TQ * t - WIN + 126),
                            0, S - 1,
                        ),
                    ]
                )
                for t in range(NT)
            ]
        ).astype(np.int64)
        x_ext_t = np.ascontiguousarray(x[b, tok_ids, :].T).astype(NPBF16)
        xt2 = np.ascontiguousarray(
            x_ext_t.reshape(6, 2, 128, EXT).transpose(0, 2, 1, 3).reshape(
                6, 128, 2 * EXT
            )
        )
        xq_t = np.ascontiguousarray(x[b, r0 : r0 + Q, :].T).astype(NPBF16)
        xq4 = _slicemajor(xq_t.reshape(1, NF, 128, Q))[0]
        mask = _build_mask(r0, gw).astype(NPBF16)
        in_maps.append(
            {
                "xt2": xt2,
                "xq4": xq4,
                "wqkv4": wqkv4,
                "wout4": wout4,
                "bcol": bcol,
                "bvb": bvb,
                "ones_sq": ones_sq,
                "mask": mask,
            }
        )
    return in_maps


def get_program():
    if "nc" not in _CACHE:
        _CACHE["nc"] = _build_program()
    return _CACHE["nc"]


def run(inputs, **spmd_kwargs):
    """Run the SPMD kernel; returns (y [B,S,DIM] fp32, BassKernelResults)."""
    x = np.asarray(inputs["x"], np.float32)
    in_maps = _prepare_in_maps(
        x,
        np.asarray(inputs["Wqkv"], np.float32),
        np.asarray(inputs["bqkv"], np.float32),
        np.asarray(inputs["Wout"], np.float32),
        np.asarray(inputs["bout"], np.float32),
        np.asarray(inputs["group_scale"], np.float32),
    )
    nc = get_program()
    res = bass_utils.run_bass_kernel_spmd(
        nc, in_maps, core_ids=list(range(NCORES)), **spmd_kwargs
    )
    y = np.empty((B, S, DIM), np.float32)
    for core in range(NCORES):
        b, sc = divmod(core, SCHUNKS)
        y[b, sc * Q : (sc + 1) * Q, :] = res.results[core]["yt"].T.astype(np.float32)
    return y, res


def kernel(**inputs):
    y, _ = run(inputs)
    return y
